# revision 22
# baseline (speedup 1.0000x reference)
"""Trainium2 Bass kernel for CRF NLL loss (nn_CRF_71571335021248).

Strategy
--------
Data-parallel over batch B=128 across 8 cores (16 sequences per core).

The forward-algorithm logsumexp scan is reformulated in exp space:
    sigma_t = (E^T sigma_{t-1}) * e_t          E = exp(trans), e_t = exp(x_t)
so each scan step is ONE PE matmul (stationary E, moving [96,16] state)
plus ONE vector-engine elementwise multiply (which also evacuates PSUM).
Host-side we subtract the per-(b,t) logsumexp of the emissions before
sending them; the CRF NLL is exactly invariant under per-timestep emission
shifts, and the shifted recursion has ~zero log-growth per step, so fp32
never overflows and no renormalization instructions are needed.

The sequential chain is halved by running the forward recursion for
t in [0, T/2-1] and the backward (beta) recursion for t in [T-1, T/2]
simultaneously, then combining:  Z = sigma_m^T E (e_{m+1} * beta_{m+1}).

Written in raw Bass (explicit semaphores): the DVE/PE instruction
encodings only fit ONE semaphore wait per instruction, so every
instruction is constructed with at most one wait, with standalone waits
only on the SP sequencer and tiny carrier copies on DVE.

The gold-path score (point + transition gathers) is computed host-side;
the device computes the log-partition function, which dominates the
compute/memory cost.
"""

import numpy as np

B, L = 128, 96
T_FULL = 1024
N_CORES = 8
BL = B // N_CORES  # 16 sequences per core
N_CHUNKS = 8
CHUNK_ORDER = [0, 7, 1, 6, 2, 5, 3, 4]

_PROGRAM_CACHE: dict = {}


def _build_program(T=T_FULL):
    from contextlib import ExitStack

    import concourse.bass as bass
    from concourse import mybir

    f32 = mybir.dt.float32
    Exp = mybir.ActivationFunctionType.Exp
    Ln = mybir.ActivationFunctionType.Ln

    csz = T // N_CHUNKS
    MID = T // 2 - 1  # number of scan steps in each direction

    nc = bass.Bass()
    xs = nc.dram_tensor("xs", [L, BL, T], f32, kind="ExternalInput")
    trs = nc.dram_tensor("trs", [L, L], f32, kind="ExternalInput")
    trst = nc.dram_tensor("trst", [L, L], f32, kind="ExternalInput")
    ones_in = nc.dram_tensor("ones", [L, 1], f32, kind="ExternalInput")
    out = nc.dram_tensor("out", [1, BL], f32, kind="ExternalOutput")

    es = ExitStack()
    with es:
        sem = lambda name: es.enter_context(nc.semaphore(name))
        sbuf = lambda name, shape: es.enter_context(nc.sbuf_tensor(name, shape, f32))
        psum = lambda name, shape: es.enter_context(nc.psum_tensor(name, shape, f32))

        dma_m = sem("dma_m")
        dma_x = [sem(f"dma_x{c}") for c in range(N_CHUNKS)]
        s_act = sem("s_act")
        s_pef = sem("s_pef")
        s_peb = sem("s_peb")
        s_pez = sem("s_pez")
        s_dvf = sem("s_dvf")
        s_dvb = sem("s_dvb")
        s_fin = sem("s_fin")

        TR = sbuf("TR", [L, L])
        TRT = sbuf("TRT", [L, L])
        E = sbuf("E", [L, L])
        ET = sbuf("ET", [L, L])
        ONESC = sbuf("ONESC", [L, 1])
        XT = sbuf("XT", [L, BL, T])
        EX = sbuf("EX", [L, BL, T])
        SIG = [sbuf("SIG0", [L, BL]), sbuf("SIG1", [L, BL])]
        U = [sbuf("U0", [L, BL]), sbuf("U1", [L, BL])]
        W = sbuf("W", [L, BL])
        LNZ = sbuf("LNZ", [1, BL])
        DUM = sbuf("DUM", [1, 16])

        PSF = [psum("PSF0", [L, BL]), psum("PSF1", [L, BL])]
        PSB = [psum("PSB0", [L, BL]), psum("PSB1", [L, BL])]
        PV = psum("PV", [L, BL])
        PZ = psum("PZ", [1, BL])

        # s_act milestone after chunk c is fully exp'd:
        act_after = {c: 2 + i for i, c in enumerate(CHUNK_ORDER)}

        with nc.Block() as block:

            @block.sync
            def _(sp):
                sp.dma_start(out=TR[:], in_=trs[:, :]).then_inc(dma_m, 16)
                sp.dma_start(out=TRT[:], in_=trst[:, :]).then_inc(dma_m, 16)
                sp.dma_start(out=ONESC[:], in_=ones_in[:, :]).then_inc(dma_m, 16)
                for ci in CHUNK_ORDER:
                    t0, t1 = ci * csz, (ci + 1) * csz
                    for b in range(BL):
                        sp.dma_start(
                            out=XT[:, b, t0:t1], in_=xs[:, b, t0:t1]
                        ).then_inc(dma_x[ci], 16)
                sp.wait_ge(s_fin, 1)
                sp.dma_start(out=out[:, :], in_=LNZ[:]).then_inc(dma_m, 16)
                sp.wait_ge(dma_m, 64)

            @block.scalar
            def _(act):
                act.activation(E[:], TR[:], Exp)._wait_ge(dma_m, 48)
                act.activation(ET[:], TRT[:], Exp).then_inc(s_act, 1)
                for ci in CHUNK_ORDER:
                    t0, t1 = ci * csz, (ci + 1) * csz
                    for b in range(BL):
                        ins = act.activation(EX[:, b, t0:t1], XT[:, b, t0:t1], Exp)
                        if b == 0:
                            ins._wait_ge(dma_x[ci], 16 * BL)
                        if b == BL - 1:
                            ins.then_inc(s_act, 1)
                act.activation(LNZ[:], PZ[:], Ln)._wait_ge(s_pez, 1).then_inc(
                    s_fin, 1
                )

            @block.tensor
            def _(pe):
                for k in range(1, MID + 1):
                    rf = EX[:, :, 0] if k == 1 else SIG[(k - 1) % 2][:]
                    mf = pe.matmul(
                        PSF[k % 2][:], lhsT=E[:], rhs=rf, start=True, stop=True
                    )
                    if k == 1:
                        mf._wait_ge(s_act, 2)
                    else:
                        mf._wait_ge(s_dvf, k - 1)
                    mf.then_inc(s_pef, 1)

                    rb = EX[:, :, T - 1] if k == 1 else U[(k - 1) % 2][:]
                    mb = pe.matmul(
                        PSB[k % 2][:], lhsT=ET[:], rhs=rb, start=True, stop=True
                    )
                    if k == 1:
                        mb._wait_ge(s_act, 3)
                    else:
                        mb._wait_ge(s_dvb, k - 1)
                    mb.then_inc(s_peb, 1)

                # v = E @ u_mid  (one more gamma-free application of E)
                pe.matmul(
                    PV[:], lhsT=ET[:], rhs=U[MID % 2][:], start=True, stop=True
                )._wait_ge(s_dvb, MID).then_inc(s_peb, 1)
                # z = ones^T (sigma_mid * v)
                pe.matmul(
                    PZ[:], lhsT=ONESC[:], rhs=W[:], start=True, stop=True
                )._wait_ge(s_dvf, MID + 1).then_inc(s_pez, 1)

            @block.vector
            def _(dv):
                ndum = 0
                for k in range(1, MID + 1):
                    if k % csz == 0:
                        cf = act_after[k // csz]
                        dv.tensor_copy(
                            DUM[:, ndum:ndum + 1], EX[0:1, 0, 0:1]
                        )._wait_ge(s_act, cf)
                        ndum += 1
                        cb = act_after[(T - 1 - k) // csz]
                        dv.tensor_copy(
                            DUM[:, ndum:ndum + 1], EX[0:1, 0, 0:1]
                        )._wait_ge(s_act, cb)
                        ndum += 1
                    dv.tensor_mul(
                        SIG[k % 2][:], PSF[k % 2][:], EX[:, :, k]
                    )._wait_ge(s_pef, k).then_inc(s_dvf, 1)
                    dv.tensor_mul(
                        U[k % 2][:], PSB[k % 2][:], EX[:, :, T - 1 - k]
                    )._wait_ge(s_peb, k).then_inc(s_dvb, 1)

                dv.tensor_mul(W[:], PV[:], SIG[MID % 2][:])._wait_ge(
                    s_peb, MID + 1
                ).then_inc(s_dvf, 1)

    return nc


def _run_cores(nc, in_maps):
    from concourse.bass_utils import run_bass_kernel_spmd

    return run_bass_kernel_spmd(nc, in_maps, list(range(len(in_maps)))).results


def make_in_maps(inputs):
    """Shift + transpose emissions; returns (in_maps, per-(b,t) shifts c)."""
    x = np.ascontiguousarray(np.asarray(inputs, dtype=np.float32))
    tr = _PROGRAM_CACHE["tr"]

    # Per-(b,t) logsumexp shift of the emissions (NLL is invariant).
    xm = x.max(axis=2, keepdims=True)
    c = (np.log(np.sum(np.exp(x - xm), axis=2, keepdims=True)) + xm).astype(np.float32)
    xsh = (x - c).astype(np.float32)

    ones = np.ones((L, 1), dtype=np.float32)
    in_maps = [
        {
            "xs": np.ascontiguousarray(
                np.transpose(xsh[ci * BL:(ci + 1) * BL], (2, 0, 1))
            ),
            "trs": tr,
            "trst": np.ascontiguousarray(tr.T),
            "ones": ones,
        }
        for ci in range(N_CORES)
    ]
    return in_maps, c


def finish(res, inputs, labels_idx, trans, c):
    """Combine device log-partition outputs with host-side gold scores."""
    x = np.asarray(inputs)
    lab = np.asarray(labels_idx)
    tr = np.asarray(trans)
    lnz = np.concatenate([np.asarray(r["out"]).reshape(BL) for r in res])  # [B]

    log_norm = lnz.astype(np.float64) + c.astype(np.float64).sum(axis=1)[:, 0]
    lab64 = lab.astype(np.int64)
    xg = np.take_along_axis(x, lab64[..., None], axis=2)[..., 0].astype(np.float64)
    point = xg.sum(axis=1)
    trans_sc = tr[lab64[:, :-1], lab64[:, 1:]].astype(np.float64).sum(axis=1)
    return (log_norm - point - trans_sc)[:, None].astype(np.float32)


def kernel(inputs, labels_idx, trans):
    if "nc" not in _PROGRAM_CACHE:
        _PROGRAM_CACHE["nc"] = _build_program()
    _PROGRAM_CACHE["tr"] = np.ascontiguousarray(np.asarray(trans, dtype=np.float32))
    nc = _PROGRAM_CACHE["nc"]

    in_maps, c = make_in_maps(inputs)
    res = _run_cores(nc, in_maps)
    return finish(res, inputs, labels_idx, trans, c)


# revision 28
# speedup vs baseline: 1.4441x; 1.4441x over previous
"""Trainium2 Bass kernel for CRF NLL loss (nn_CRF_71571335021248).

Strategy
--------
Data-parallel over batch B=128 across 8 cores (16 sequences per core).

The forward-algorithm logsumexp scan is reformulated in exp space:
    sigma_t = (E^T sigma_{t-1}) * e_t          E = exp(trans), e_t = exp(x_t)
so each scan step is ONE PE matmul (stationary E, moving [96,16] state)
plus ONE vector-engine elementwise multiply (which also evacuates PSUM).
Host-side we subtract the per-(b,t) logsumexp of the emissions before
sending them; the CRF NLL is exactly invariant under per-timestep emission
shifts, and the shifted recursion has ~zero log-growth per step, so fp32
never overflows and no renormalization instructions are needed.

The sequential chain is halved by running the forward recursion for
t in [0, T/2-1] and the backward (beta) recursion for t in [T-1, T/2]
simultaneously, then combining:  Z = sigma_m^T E (e_{m+1} * beta_{m+1}).

Written in raw Bass (explicit semaphores): the DVE/PE instruction
encodings only fit ONE semaphore wait per instruction, so every
instruction is constructed with at most one wait, with standalone waits
only on the SP sequencer and tiny carrier copies on DVE.

The gold-path score (point + transition gathers) is computed host-side;
the device computes the log-partition function, which dominates the
compute/memory cost.
"""

import numpy as np

B, L = 128, 96
T_FULL = 1024
N_CORES = 8
BL = B // N_CORES  # 16 sequences per core
N_CHUNKS = 8
CHUNK_ORDER = [0, 7, 1, 6, 2, 5, 3, 4]

_PROGRAM_CACHE: dict = {}


def _build_program(T=T_FULL):
    from contextlib import ExitStack

    import concourse.bass as bass
    from concourse import mybir

    f32 = mybir.dt.float32
    bf16 = mybir.dt.bfloat16
    Exp = mybir.ActivationFunctionType.Exp
    Ln = mybir.ActivationFunctionType.Ln

    csz = T // N_CHUNKS
    MID = T // 2 - 1  # number of scan steps in each direction

    nc = bass.Bass()
    xs = nc.dram_tensor("xs", [L, BL, T], f32, kind="ExternalInput")
    trs = nc.dram_tensor("trs", [L, L], f32, kind="ExternalInput")
    trst = nc.dram_tensor("trst", [L, L], f32, kind="ExternalInput")
    ones_in = nc.dram_tensor("ones", [L, 1], bf16, kind="ExternalInput")
    out = nc.dram_tensor("out", [1, BL], f32, kind="ExternalOutput")

    es = ExitStack()
    with es:
        sem = lambda name: es.enter_context(nc.semaphore(name))
        sbuf = lambda name, shape, dt=f32: es.enter_context(
            nc.sbuf_tensor(name, shape, dt)
        )
        psum = lambda name, shape: es.enter_context(nc.psum_tensor(name, shape, f32))

        dma_m = sem("dma_m")
        dma_x = [sem(f"dma_x{c}") for c in range(N_CHUNKS)]
        s_act = sem("s_act")
        s_pef = sem("s_pef")
        s_peb = sem("s_peb")
        s_pez = sem("s_pez")
        s_dvf = sem("s_dvf")
        s_dvb = sem("s_dvb")
        s_fin = sem("s_fin")

        TR = sbuf("TR", [L, L])
        TRT = sbuf("TRT", [L, L])
        E = sbuf("E", [L, L], bf16)
        ET = sbuf("ET", [L, L], bf16)
        ONESC = sbuf("ONESC", [L, 1], bf16)
        XT = sbuf("XT", [L, BL, T])
        EX = sbuf("EX", [L, BL, T], bf16)
        SIG = [sbuf("SIG0", [L, BL], bf16), sbuf("SIG1", [L, BL], bf16)]
        U = [sbuf("U0", [L, BL], bf16), sbuf("U1", [L, BL], bf16)]
        W = sbuf("W", [L, BL], bf16)
        LNZ = sbuf("LNZ", [1, BL])
        DUM = sbuf("DUM", [1, 16], bf16)

        PSF = [psum("PSF0", [L, BL]), psum("PSF1", [L, BL])]
        PSB = [psum("PSB0", [L, BL]), psum("PSB1", [L, BL])]
        PV = psum("PV", [L, BL])
        PZ = psum("PZ", [1, BL])

        # s_act milestone after chunk c is fully exp'd:
        act_after = {c: 2 + i for i, c in enumerate(CHUNK_ORDER)}

        with nc.Block() as block:

            @block.sync
            def _(sp):
                sp.dma_start(out=TR[:], in_=trs[:, :]).then_inc(dma_m, 16)
                sp.dma_start(out=TRT[:], in_=trst[:, :]).then_inc(dma_m, 16)
                sp.dma_start(out=ONESC[:], in_=ones_in[:, :]).then_inc(dma_m, 16)
                sp.wait_ge(s_fin, 1)
                sp.dma_start(out=out[:, :], in_=LNZ[:]).then_inc(dma_m, 16)
                sp.wait_ge(dma_m, 64)

            @block.gpsimd
            def _(gp):
                for ci in CHUNK_ORDER:
                    t0, t1 = ci * csz, (ci + 1) * csz
                    for b in range(BL):
                        gp.dma_start(
                            out=XT[:, b, t0:t1], in_=xs[:, b, t0:t1]
                        ).then_inc(dma_x[ci], 16)

            @block.scalar
            def _(act):
                act.activation(E[:], TR[:], Exp)._wait_ge(dma_m, 48)
                act.activation(ET[:], TRT[:], Exp).then_inc(s_act, 1)
                for ci in CHUNK_ORDER:
                    t0, t1 = ci * csz, (ci + 1) * csz
                    for b in range(BL):
                        ins = act.activation(EX[:, b, t0:t1], XT[:, b, t0:t1], Exp)
                        if b == 0:
                            ins._wait_ge(dma_x[ci], 16 * BL)
                        if b == BL - 1:
                            ins.then_inc(s_act, 1)
                act.activation(LNZ[:], PZ[:], Ln)._wait_ge(s_pez, 1).then_inc(
                    s_fin, 1
                )

            @block.tensor
            def _(pe):
                for k in range(1, MID + 1):
                    rf = EX[:, :, 0] if k == 1 else SIG[(k - 1) % 2][:]
                    mf = pe.matmul(
                        PSF[k % 2][:], lhsT=E[:], rhs=rf, start=True, stop=True
                    )
                    if k == 1:
                        mf._wait_ge(s_act, 2)
                    else:
                        mf._wait_ge(s_dvf, k - 1)
                    mf.then_inc(s_pef, 1)

                    rb = EX[:, :, T - 1] if k == 1 else U[(k - 1) % 2][:]
                    mb = pe.matmul(
                        PSB[k % 2][:], lhsT=ET[:], rhs=rb, start=True, stop=True
                    )
                    if k == 1:
                        mb._wait_ge(s_act, 3)
                    else:
                        mb._wait_ge(s_dvb, k - 1)
                    mb.then_inc(s_peb, 1)

                # v = E @ u_mid  (one more gamma-free application of E)
                pe.matmul(
                    PV[:], lhsT=ET[:], rhs=U[MID % 2][:], start=True, stop=True
                )._wait_ge(s_dvb, MID).then_inc(s_peb, 1)
                # z = ones^T (sigma_mid * v)
                pe.matmul(
                    PZ[:], lhsT=ONESC[:], rhs=W[:], start=True, stop=True
                )._wait_ge(s_dvf, MID + 1).then_inc(s_pez, 1)

            @block.vector
            def _(dv):
                ndum = 0
                for k in range(1, MID + 1):
                    if k % csz == 0:
                        cf = act_after[k // csz]
                        dv.tensor_copy(
                            DUM[:, ndum:ndum + 1], EX[0:1, 0, 0:1]
                        )._wait_ge(s_act, cf)
                        ndum += 1
                        cb = act_after[(T - 1 - k) // csz]
                        dv.tensor_copy(
                            DUM[:, ndum:ndum + 1], EX[0:1, 0, 0:1]
                        )._wait_ge(s_act, cb)
                        ndum += 1
                    dv.tensor_mul(
                        SIG[k % 2][:], PSF[k % 2][:], EX[:, :, k]
                    )._wait_ge(s_pef, k).then_inc(s_dvf, 1)
                    dv.tensor_mul(
                        U[k % 2][:], PSB[k % 2][:], EX[:, :, T - 1 - k]
                    )._wait_ge(s_peb, k).then_inc(s_dvb, 1)

                dv.tensor_mul(W[:], PV[:], SIG[MID % 2][:])._wait_ge(
                    s_peb, MID + 1
                ).then_inc(s_dvf, 1)

    return nc


def _run_cores(nc, in_maps):
    from concourse.bass_utils import run_bass_kernel_spmd

    return run_bass_kernel_spmd(nc, in_maps, list(range(len(in_maps)))).results


def make_in_maps(inputs):
    """Shift + transpose emissions; returns (in_maps, per-(b,t) shifts c)."""
    x = np.ascontiguousarray(np.asarray(inputs, dtype=np.float32))
    tr = _PROGRAM_CACHE["tr"]

    # Per-(b,t) logsumexp shift of the emissions (NLL is invariant).
    xm = x.max(axis=2, keepdims=True)
    c = (np.log(np.sum(np.exp(x - xm), axis=2, keepdims=True)) + xm).astype(np.float32)
    xsh = (x - c).astype(np.float32)

    import ml_dtypes
    ones = np.ones((L, 1), dtype=ml_dtypes.bfloat16)
    in_maps = [
        {
            "xs": np.ascontiguousarray(
                np.transpose(xsh[ci * BL:(ci + 1) * BL], (2, 0, 1))
            ),
            "trs": tr,
            "trst": np.ascontiguousarray(tr.T),
            "ones": ones,
        }
        for ci in range(N_CORES)
    ]
    return in_maps, c


def finish(res, inputs, labels_idx, trans, c):
    """Combine device log-partition outputs with host-side gold scores."""
    x = np.asarray(inputs)
    lab = np.asarray(labels_idx)
    tr = np.asarray(trans)
    lnz = np.concatenate([np.asarray(r["out"]).reshape(BL) for r in res])  # [B]

    log_norm = lnz.astype(np.float64) + c.astype(np.float64).sum(axis=1)[:, 0]
    lab64 = lab.astype(np.int64)
    xg = np.take_along_axis(x, lab64[..., None], axis=2)[..., 0].astype(np.float64)
    point = xg.sum(axis=1)
    trans_sc = tr[lab64[:, :-1], lab64[:, 1:]].astype(np.float64).sum(axis=1)
    return (log_norm - point - trans_sc)[:, None].astype(np.float32)


def kernel(inputs, labels_idx, trans):
    if "nc" not in _PROGRAM_CACHE:
        _PROGRAM_CACHE["nc"] = _build_program()
    _PROGRAM_CACHE["tr"] = np.ascontiguousarray(np.asarray(trans, dtype=np.float32))
    nc = _PROGRAM_CACHE["nc"]

    in_maps, c = make_in_maps(inputs)
    res = _run_cores(nc, in_maps)
    return finish(res, inputs, labels_idx, trans, c)


# revision 29
# speedup vs baseline: 1.6618x; 1.1507x over previous
"""Trainium2 Bass kernel for CRF NLL loss (nn_CRF_71571335021248).

Strategy
--------
Data-parallel over batch B=128 across 8 cores (16 sequences per core).

The forward-algorithm logsumexp scan is reformulated in exp space:
    sigma_t = (E^T sigma_{t-1}) * e_t          E = exp(trans), e_t = exp(x_t)
so each scan step is ONE PE matmul (stationary E, moving [96,16] state)
plus ONE vector-engine elementwise multiply (which also evacuates PSUM).
Host-side we subtract the per-(b,t) logsumexp of the emissions before
sending them; the CRF NLL is exactly invariant under per-timestep emission
shifts, and the shifted recursion has ~zero log-growth per step, so fp32
never overflows and no renormalization instructions are needed.

The sequential chain is halved by running the forward recursion for
t in [0, T/2-1] and the backward (beta) recursion for t in [T-1, T/2]
simultaneously, then combining:  Z = sigma_m^T E (e_{m+1} * beta_{m+1}).

Written in raw Bass (explicit semaphores): the DVE/PE instruction
encodings only fit ONE semaphore wait per instruction, so every
instruction is constructed with at most one wait, with standalone waits
only on the SP sequencer and tiny carrier copies on DVE.

The gold-path score (point + transition gathers) is computed host-side;
the device computes the log-partition function, which dominates the
compute/memory cost.
"""

import numpy as np

B, L = 128, 96
T_FULL = 1024
N_CORES = 8
BL = B // N_CORES  # 16 sequences per core
N_CHUNKS = 8
CHUNK_ORDER = [0, 7, 1, 6, 2, 5, 3, 4]

_PROGRAM_CACHE: dict = {}


def _build_program(T=T_FULL):
    from contextlib import ExitStack

    import concourse.bass as bass
    from concourse import mybir

    f32 = mybir.dt.float32
    bf16 = mybir.dt.bfloat16
    Exp = mybir.ActivationFunctionType.Exp
    Ln = mybir.ActivationFunctionType.Ln

    csz = T // N_CHUNKS
    MID = T // 2 - 1  # number of scan steps in each direction

    nc = bass.Bass()
    xs = nc.dram_tensor("xs", [L, BL, T], f32, kind="ExternalInput")
    trs = nc.dram_tensor("trs", [L, L], f32, kind="ExternalInput")
    trst = nc.dram_tensor("trst", [L, L], f32, kind="ExternalInput")
    ones_in = nc.dram_tensor("ones", [L, 1], bf16, kind="ExternalInput")
    out = nc.dram_tensor("out", [1, BL], f32, kind="ExternalOutput")

    es = ExitStack()
    with es:
        sem = lambda name: es.enter_context(nc.semaphore(name))
        sbuf = lambda name, shape, dt=f32: es.enter_context(
            nc.sbuf_tensor(name, shape, dt)
        )
        psum = lambda name, shape: es.enter_context(nc.psum_tensor(name, shape, f32))

        dma_m = sem("dma_m")
        dma_x = [sem(f"dma_x{c}") for c in range(N_CHUNKS)]
        s_act = sem("s_act")
        s_pef = sem("s_pef")
        s_peb = sem("s_peb")
        s_pez = sem("s_pez")
        s_dvf = sem("s_dvf")
        s_dvb = sem("s_dvb")
        s_fin = sem("s_fin")

        TR = sbuf("TR", [L, L])
        TRT = sbuf("TRT", [L, L])
        E = sbuf("E", [L, L], bf16)
        ET = sbuf("ET", [L, L], bf16)
        ONESC = sbuf("ONESC", [L, 1], bf16)
        XT = sbuf("XT", [L, BL, T])
        EX = sbuf("EX", [L, BL, T], bf16)
        SIG = [sbuf("SIG0", [L, BL], bf16), sbuf("SIG1", [L, BL], bf16)]
        U = [sbuf("U0", [L, BL], bf16), sbuf("U1", [L, BL], bf16)]
        W = sbuf("W", [L, BL], bf16)
        LNZ = sbuf("LNZ", [1, BL])
        DUM = sbuf("DUM", [1, 16], bf16)

        PSF = [psum("PSF0", [L, BL]), psum("PSF1", [L, BL])]
        PSB = [psum("PSB0", [L, BL]), psum("PSB1", [L, BL])]
        PV = psum("PV", [L, BL])
        PZ = psum("PZ", [1, BL])

        # s_act milestone after chunk c is fully exp'd:
        act_after = {c: 2 + i for i, c in enumerate(CHUNK_ORDER)}

        with nc.Block() as block:

            @block.sync
            def _(sp):
                sp.dma_start(out=TR[:], in_=trs[:, :]).then_inc(dma_m, 16)
                sp.dma_start(out=TRT[:], in_=trst[:, :]).then_inc(dma_m, 16)
                sp.dma_start(out=ONESC[:], in_=ones_in[:, :]).then_inc(dma_m, 16)
                sp.wait_ge(s_fin, 1)
                sp.dma_start(out=out[:, :], in_=LNZ[:]).then_inc(dma_m, 16)
                sp.wait_ge(dma_m, 64)

            @block.gpsimd
            def _(gp):
                for ci in CHUNK_ORDER:
                    t0, t1 = ci * csz, (ci + 1) * csz
                    for b in range(BL):
                        gp.dma_start(
                            out=XT[:, b, t0:t1], in_=xs[:, b, t0:t1]
                        ).then_inc(dma_x[ci], 16)

            @block.scalar
            def _(act):
                act.activation(E[:], TR[:], Exp)._wait_ge(dma_m, 48)
                act.activation(ET[:], TRT[:], Exp).then_inc(s_act, 1)
                for ci in CHUNK_ORDER:
                    t0, t1 = ci * csz, (ci + 1) * csz
                    for b in range(BL):
                        ins = act.activation(EX[:, b, t0:t1], XT[:, b, t0:t1], Exp)
                        if b == 0:
                            ins._wait_ge(dma_x[ci], 16 * BL)
                        if b == BL - 1:
                            ins.then_inc(s_act, 1)
                act.activation(LNZ[:], PZ[:], Ln)._wait_ge(s_pez, 1).then_inc(
                    s_fin, 1
                )

            @block.tensor
            def _(pe):
                def mm(out_ap, lhsT, rhs):
                    ins = pe.matmul(out_ap, lhsT=lhsT, rhs=rhs, start=True, stop=True)
                    ins.ins.ldweights = False
                    return ins

                for k in range(1, MID + 1):
                    ldw = pe.ldweights(E[:])
                    if k == 1:
                        ldw._wait_ge(s_act, 2)
                    rf = EX[:, :, 0] if k == 1 else SIG[(k - 1) % 2][:]
                    mf = mm(PSF[k % 2][:], E[:], rf)
                    if k > 1:
                        mf._wait_ge(s_dvf, k - 1)
                    mf.then_inc(s_pef, 1)

                    ldwb = pe.ldweights(ET[:])
                    if k == 1:
                        ldwb._wait_ge(s_act, 3)
                    rb = EX[:, :, T - 1] if k == 1 else U[(k - 1) % 2][:]
                    mb = mm(PSB[k % 2][:], ET[:], rb)
                    if k > 1:
                        mb._wait_ge(s_dvb, k - 1)
                    mb.then_inc(s_peb, 1)

                # v = E @ u_mid
                pe.ldweights(ET[:])
                mm(PV[:], ET[:], U[MID % 2][:])._wait_ge(s_dvb, MID).then_inc(
                    s_peb, 1
                )
                # z = ones^T (sigma_mid * v)
                pe.ldweights(ONESC[:])
                mm(PZ[:], ONESC[:], W[:])._wait_ge(s_dvf, MID + 1).then_inc(s_pez, 1)

            @block.vector
            def _(dv):
                ndum = 0
                for k in range(1, MID + 1):
                    if k % csz == 0:
                        cf = act_after[k // csz]
                        dv.tensor_copy(
                            DUM[:, ndum:ndum + 1], EX[0:1, 0, 0:1]
                        )._wait_ge(s_act, cf)
                        ndum += 1
                        cb = act_after[(T - 1 - k) // csz]
                        dv.tensor_copy(
                            DUM[:, ndum:ndum + 1], EX[0:1, 0, 0:1]
                        )._wait_ge(s_act, cb)
                        ndum += 1
                    dv.tensor_mul(
                        SIG[k % 2][:], PSF[k % 2][:], EX[:, :, k]
                    )._wait_ge(s_pef, k).then_inc(s_dvf, 1)
                    dv.tensor_mul(
                        U[k % 2][:], PSB[k % 2][:], EX[:, :, T - 1 - k]
                    )._wait_ge(s_peb, k).then_inc(s_dvb, 1)

                dv.tensor_mul(W[:], PV[:], SIG[MID % 2][:])._wait_ge(
                    s_peb, MID + 1
                ).then_inc(s_dvf, 1)

    return nc


def _run_cores(nc, in_maps):
    from concourse.bass_utils import run_bass_kernel_spmd

    return run_bass_kernel_spmd(nc, in_maps, list(range(len(in_maps)))).results


def make_in_maps(inputs):
    """Shift + transpose emissions; returns (in_maps, per-(b,t) shifts c)."""
    x = np.ascontiguousarray(np.asarray(inputs, dtype=np.float32))
    tr = _PROGRAM_CACHE["tr"]

    # Per-(b,t) logsumexp shift of the emissions (NLL is invariant).
    xm = x.max(axis=2, keepdims=True)
    c = (np.log(np.sum(np.exp(x - xm), axis=2, keepdims=True)) + xm).astype(np.float32)
    xsh = (x - c).astype(np.float32)

    import ml_dtypes
    ones = np.ones((L, 1), dtype=ml_dtypes.bfloat16)
    in_maps = [
        {
            "xs": np.ascontiguousarray(
                np.transpose(xsh[ci * BL:(ci + 1) * BL], (2, 0, 1))
            ),
            "trs": tr,
            "trst": np.ascontiguousarray(tr.T),
            "ones": ones,
        }
        for ci in range(N_CORES)
    ]
    return in_maps, c


def finish(res, inputs, labels_idx, trans, c):
    """Combine device log-partition outputs with host-side gold scores."""
    x = np.asarray(inputs)
    lab = np.asarray(labels_idx)
    tr = np.asarray(trans)
    lnz = np.concatenate([np.asarray(r["out"]).reshape(BL) for r in res])  # [B]

    log_norm = lnz.astype(np.float64) + c.astype(np.float64).sum(axis=1)[:, 0]
    lab64 = lab.astype(np.int64)
    xg = np.take_along_axis(x, lab64[..., None], axis=2)[..., 0].astype(np.float64)
    point = xg.sum(axis=1)
    trans_sc = tr[lab64[:, :-1], lab64[:, 1:]].astype(np.float64).sum(axis=1)
    return (log_norm - point - trans_sc)[:, None].astype(np.float32)


def kernel(inputs, labels_idx, trans):
    if "nc" not in _PROGRAM_CACHE:
        _PROGRAM_CACHE["nc"] = _build_program()
    _PROGRAM_CACHE["tr"] = np.ascontiguousarray(np.asarray(trans, dtype=np.float32))
    nc = _PROGRAM_CACHE["nc"]

    in_maps, c = make_in_maps(inputs)
    res = _run_cores(nc, in_maps)
    return finish(res, inputs, labels_idx, trans, c)
